# revision 1
# baseline (speedup 1.0000x reference)
"""Trainium2 Bass kernel for nn_Biholomorphic_k8.

Computes, for z in C^5 (given as z_re, z_im [256, 5] f32):
  zz   = degree-8 monomials of z            [256, 495] complex
  zzbar = zz[:, IU] * conj(zz[:, JU])       upper-triangular pairs
  out  = concat(re(zzbar), im(zzbar strict)) -> [256, 245025] f32

Device strategy (8 cores, pure batch data-parallel, 32 batch rows/core):
  - batch replicated 4x across the 128 SBUF partitions (partition 32g+b = batch b)
  - zz built on-device via the "prepend smallest coordinate" ladder: every
    degree-d monomial block with first coord c is z_c times a contiguous
    suffix of the degree-(d-1) block, so the whole construction is
    tensor_scalar / scalar_tensor_tensor ops on interleaved (re,im) tiles.
  - main loop: tick `it` computes triangle rows i = 4*it+g on partition
    group g over the shared column suffix [4*it, 495):
        re row = ZR_j*zr_i + ZI_j*zi_i ;  im row = ZI_j*zr_i - ZR_j*zi_i
    with per-partition scalars zr_i, zi_i; the shared products ZRI*zi_i go
    on the scalar engine (activation Copy w/ scale), the two fused
    scalar_tensor_tensor ops on the vector engine.
  - results are packed *exactly* (no padding) into [128, ~4K] tiles and
    streamed to HBM with ~16 large DMAs (small per-row DMAs measured at
    48GB/s vs 336GB/s for large ones). Host applies the pure-permutation
    unpack to the required layout (no arithmetic on host).
"""

import itertools
import math
import os
import sys

import numpy as np

if "/opt/trn_rl_repo" not in sys.path:
    sys.path.insert(0, "/opt/trn_rl_repo")

N_COORD = 5
DEGREE = 8
N_MONO = 495          # C(5+8-1, 8)
N_PAIRS = 122760      # 495*496/2
OUT_W = 245025        # pairs + strict
B = 256
B_CORE = 32           # batch rows per core
N_CORES = 8
TICKS = 124           # ceil(495/4)
TCAP = 3168           # staging tile width (fp32 cols per partition)

# ---- combinatorial tables (computed, not read from files) ----
M = [0] * (DEGREE + 1)          # number of degree-d monomials
for d in range(1, DEGREE + 1):
    M[d] = math.comb(N_COORD + d - 1, d)
# SOFF[d][c] = index of first degree-d sorted tuple whose min coord >= c
SOFF = [[0] * (N_COORD + 1) for _ in range(DEGREE + 1)]
for d in range(1, DEGREE + 1):
    for c in range(N_COORD + 1):
        SOFF[d][c] = M[d] - math.comb(N_COORD - c + d - 1, d)

OFF_RE = np.concatenate([[0], np.cumsum(495 - np.arange(495))]).astype(np.int64)
OFF_IM = np.concatenate([[0], np.cumsum(494 - np.arange(494))]).astype(np.int64)

TICK_L = [495 - 4 * it for it in range(TICKS)]
TOT = sum(2 * L for L in TICK_L)  # 61752 blob cols per partition


def _plan_tmp_engines():
    """Assign each tick's tmp (tensor_scalar) op to ACT or DVE to balance.

    Measured per-op costs (ns): DVE stt(2L): 132 + 2.08*L; DVE ts(2L):
    222 + 1.03*L; ACT ts(2L): 300 + 1.67*L.
    """
    act = [True] * TICKS
    # + measured non-main-loop engine load (ladder/prep/waits)
    act_ns = sum(287 + 1.67 * L for L in TICK_L)
    dve_ns = sum(171 + 2.08 * L for L in TICK_L) + 10000
    order = sorted(range(TICKS), key=lambda it: TICK_L[it])
    for it in order:
        L = TICK_L[it]
        d_act = 287 + 1.67 * L
        d_dve = 150 + 1.03 * L
        if act_ns > dve_ns + d_dve:
            act[it] = False
            act_ns -= d_act
            dve_ns += d_dve
    return act


def _build_unpack_index():
    """outcol[g][x] = output column fed by blob[32g+b, x], or -1 (junk).

    Each tick's 2L-wide piece is interleaved: col 2k -> re(i, j=4it+k),
    col 2k+1 -> im(i, j).
    """
    outcol = np.full((4, TOT), -1, dtype=np.int64)
    o = 0
    for it in range(TICKS):
        L = TICK_L[it]
        for g in range(4):
            i = 4 * it + g
            if i >= N_MONO:
                continue
            seg = 495 - i  # = L - g
            outcol[g, o + 2 * g:o + 2 * L:2] = OFF_RE[i] + np.arange(seg)
            if i < 494:
                outcol[g, o + 2 * (g + 1) + 1:o + 2 * L:2] = (
                    N_PAIRS + OFF_IM[i] + np.arange(494 - i))
        o += 2 * L
    assert o == TOT
    return outcol


_OUTCOL = _build_unpack_index()
_PROGRAM = None


def _build_program():
    import concourse.bacc as bacc
    import concourse.mybir as mybir
    from concourse.tile import TileContext

    f32 = mybir.dt.float32
    mult = mybir.AluOpType.mult
    add = mybir.AluOpType.add
    sub = mybir.AluOpType.subtract

    nc = bacc.Bacc(None)
    zin = nc.dram_tensor("zin", [128, 2 * N_COORD], f32, kind="ExternalInput")
    blob = nc.dram_tensor("blob", [128, TOT], f32, kind="ExternalOutput")

    tmp_on_act = _plan_tmp_engines()

    with TileContext(nc) as tc:
        with (
            tc.tile_pool(name="const", bufs=1) as cpool,
            tc.tile_pool(name="lad", bufs=1) as lpool,
            tc.tile_pool(name="tmp", bufs=10) as tpool,
            tc.tile_pool(name="stage", bufs=4) as opool,
        ):
            z1 = cpool.tile([128, 2 * N_COORD], f32)
            nc.sync.dma_start(z1[:], zin[:])

            # ---- monomial ladder: interleaved (re, im) per degree ----
            # Tail-first block order: build blocks c = 4..1 of every degree
            # first, so the tail monomials [330, 495) finish early and the
            # reversed main loop can start under the big c=0 chain.
            deg = {1: z1}
            for d in range(2, DEGREE + 1):
                deg[d] = lpool.tile(
                    [128, 2 * M[d]], f32, name=f"deg{d}", tag=f"deg{d}")

            def ladder_block(d, c):
                prev, cur = deg[d - 1], deg[d]
                sp = SOFF[d - 1][c]
                Lc = M[d - 1] - sp
                do_ = SOFF[d][c]
                src = prev[:, 2 * sp:2 * M[d - 1]]
                src_ev = prev[:, 2 * sp:2 * M[d - 1]:2]
                src_od = prev[:, 2 * sp + 1:2 * M[d - 1]:2]
                zr = z1[:, 2 * c:2 * c + 1]
                zi = z1[:, 2 * c + 1:2 * c + 2]
                t = tpool.tile([128, 2 * M[DEGREE - 1]], f32, tag="ladtmp")
                if Lc >= 64:
                    nc.scalar.mul(t[:, 0:2 * Lc], src, zi)
                else:
                    nc.vector.tensor_scalar(t[:, 0:2 * Lc], src, zi, None, mult)
                # re' = re*zr - im*zi
                nc.vector.scalar_tensor_tensor(
                    cur[:, 2 * do_:2 * (do_ + Lc):2], src_ev, zr,
                    t[:, 1:2 * Lc:2], mult, sub)
                # im' = im*zr + re*zi
                nc.vector.scalar_tensor_tensor(
                    cur[:, 2 * do_ + 1:2 * (do_ + Lc):2], src_od, zr,
                    t[:, 0:2 * Lc:2], mult, add)

            for d in range(2, DEGREE + 1):
                for c in range(N_COORD):
                    ladder_block(d, c)
            ZRI = deg[DEGREE]  # [128, 990] interleaved degree-8 monomials

            # ---- derived arrays + scalar tables, in two chunks ----
            W = 2 * N_MONO
            ZRIc = cpool.tile([128, W], f32)
            ZRI3 = cpool.tile([128, W], f32)
            S_zr = cpool.tile([128, TICKS], f32)
            S_zi = cpool.tile([128, TICKS], f32)
            nc.vector.memset(S_zr[:], 0.0)
            nc.vector.memset(S_zi[:], 0.0)

            def prep_chunk(m0, m1, it0, it1):
                a, b = 2 * m0, 2 * m1
                nc.vector.tensor_copy(ZRIc[:, a:b - 1:2], ZRI[:, a:b - 1:2])
                nc.vector.tensor_scalar(
                    ZRIc[:, a + 1:b:2], ZRI[:, a + 1:b:2], -1.0, None, mult)
                nc.scalar.copy(ZRI3[:, a:b - 1:2], ZRI[:, a + 1:b:2])
                nc.scalar.copy(ZRI3[:, a + 1:b:2], ZRI[:, a:b - 1:2])
                for g in range(4):
                    hi = min(it1, TICKS - 1 if g == 3 else TICKS)
                    if hi <= it0:
                        continue
                    nc.vector.tensor_copy(
                        S_zr[32 * g:32 * (g + 1), it0:hi],
                        ZRI[32 * g:32 * (g + 1),
                            8 * it0 + 2 * g:8 * (hi - 1) + 2 * g + 1:8])
                    nc.scalar.copy(
                        S_zi[32 * g:32 * (g + 1), it0:hi],
                        ZRI[32 * g:32 * (g + 1),
                            8 * it0 + 2 * g + 1:8 * (hi - 1) + 2 * g + 2:8])

            prep_chunk(0, N_MONO, 0, TICKS)

            # ---- main loop ----
            o = 0
            blob_off = 0
            T = opool.tile([128, TCAP], f32, tag="T")
            for it in range(TICKS):
                L = TICK_L[it]
                base = 8 * it
                if o + 2 * L > TCAP:
                    eng = nc.sync if (blob_off // TCAP) % 2 == 0 else nc.scalar
                    eng.dma_start(blob[:, blob_off:blob_off + o], T[:, 0:o])
                    blob_off += o
                    o = 0
                    T = opool.tile([128, TCAP], f32, tag="T")
                t = tpool.tile([128, 990], f32, tag="mtmp")
                szr = S_zr[:, it:it + 1]
                szi = S_zi[:, it:it + 1]
                # tmp = (ZI_j, ZR_j) * zi_i  (interleaved)
                if tmp_on_act[it]:
                    nc.scalar.mul(t[:, 0:2 * L], ZRI3[:, base:base + 2 * L], szi)
                else:
                    nc.vector.tensor_scalar(
                        t[:, 0:2 * L], ZRI3[:, base:base + 2 * L], szi, None, mult)
                # out interleaved (re, im): (ZR_j, -ZI_j)*zr_i + tmp
                nc.vector.scalar_tensor_tensor(
                    T[:, o:o + 2 * L], ZRIc[:, base:base + 2 * L], szr,
                    t[:, 0:2 * L], mult, add)
                o += 2 * L
            nc.sync.dma_start(blob[:, blob_off:blob_off + o], T[:, 0:o])
            assert blob_off + o == TOT

    nc.compile()
    return nc


def _get_program():
    global _PROGRAM
    if _PROGRAM is None:
        _PROGRAM = _build_program()
    return _PROGRAM


LAST_EXEC_NS = None


def kernel(z_re: np.ndarray, z_im: np.ndarray) -> np.ndarray:
    global LAST_EXEC_NS
    from concourse.bass_utils import run_bass_kernel_spmd

    z_re = np.asarray(z_re, dtype=np.float32)
    z_im = np.asarray(z_im, dtype=np.float32)
    assert z_re.shape == (B, N_COORD) and z_im.shape == (B, N_COORD)

    nc = _get_program()

    in_maps = []
    for c in range(N_CORES):
        zr = z_re[c * B_CORE:(c + 1) * B_CORE]   # [32, 5]
        zi = z_im[c * B_CORE:(c + 1) * B_CORE]
        zin = np.empty((B_CORE, 2 * N_COORD), np.float32)
        zin[:, 0::2] = zr
        zin[:, 1::2] = zi
        in_maps.append({"zin": np.tile(zin, (4, 1))})  # [128, 10]

    trace = bool(os.environ.get("BIHOLO_TRACE"))
    res = run_bass_kernel_spmd(
        nc, in_maps, core_ids=list(range(N_CORES)), trace=trace)
    if trace:
        LAST_EXEC_NS = res.exec_time_ns

    out = np.empty((B, OUT_W), np.float32)
    for c in range(N_CORES):
        b = np.asarray(res.results[c]["blob"])  # [128, TOT]
        rows = slice(c * B_CORE, (c + 1) * B_CORE)
        for g in range(4):
            cols = _OUTCOL[g]
            valid = cols >= 0
            out[rows, cols[valid]] = b[32 * g:32 * (g + 1), valid]
    return out

